# revision 10
# baseline (speedup 1.0000x reference)
"""CRF mean-field (dense_cnn) Trainium2 Bass kernel — v2 (pipelined).

Math: per iteration  x = x0 + w*separable_blur(softmax(x, axis=C))
(the -I compatibility einsum is a sign flip). Core i handles sample i.

Per-core scheme (SBUF-resident, bf16 compute):
  X0[c]  [128, 2048] bf16   x0, h-blocks side by side (host-cast)
  EM[c]  [128, 2048] bf16   e = exp(x0+m); softmax turns it into q in place
  Blur on PE with banded bf16 matrices Ah/Aw (weight folded into Aw):
    pass1: o1[w,h] = sum_h' q[h',w] Ah[h',h]   (lhsT=q block, rhs=Ah)
    pass2: m[h,w]  = sum_w' o1[w',h] Aw[w',w]  (lhsT=o1 block, rhs=Aw)
  x0 rides the PE via identity matmuls into the pass-2 PSUM (iters 0..3),
  so ACT's exp(psum) directly yields e for the next iteration.

v2 scheduling (vs v1):
  - Channel software pipelining: pass1(c+1) is issued between pass1(c)'s
    copies and pass2(c), so the PE never stalls on the PSUM->SBUF drains.
  - o1 drains alternate ACT/DVE; exp done as 2x[128,1024] (psum pair).
  - den accumulated in two chains (pA ch0-9, pB ch10-18) interleaved with
    the channel loop; softmax tail (merge/recip/cast) split in spatial
    halves and first-channel muls in quarters so the PE restart after an
    iteration boundary stays under the HAM re-throttle window.
  - Last iteration: no identity matmuls; drain is DVE add of bf16 x0 with
    the psum (no f32 x0 reload), output DMA per [128,1024] half.
  - Dummy warm-up matmuls (anchored on exp(17) of the prologue) bring the
    PE HAM clock to 2.4GHz before the first real pass1.
"""

import sys

for _p in ("/opt/trn_rl_repo",):
    if _p not in sys.path:
        sys.path.insert(0, _p)

import numpy as np
import ml_dtypes

import concourse.bass as bass
from concourse import bacc
import concourse.mybir as mybir
import concourse.tile as tile
from concourse.bass_utils import run_bass_kernel_spmd
from concourse.tile_rust import add_dep_helper

F32 = mybir.dt.float32
BF16 = mybir.dt.bfloat16
P = 128
R = 5          # filter radius (FS=11)
N_CORES = 8

BF16_NP = ml_dtypes.bfloat16

EXP = mybir.ActivationFunctionType.Exp


def _conv_pieces(nb, n, sim_safe):
    """Per row-block j, list of (lo, hi, is_start) output-column windows."""
    out = []
    for j in range(nb):
        base = j * P
        hi = min(n, base + P + R)
        if j == 0:
            out.append([(0, hi, True)])
        elif not sim_safe:
            out.append([(base - R, hi, False)])
        else:
            pieces = [(base - R, base + R, False)]       # overlap: accumulate
            if base + R < hi:
                pieces.append((base + R, hi, False))     # fresh: overwrite
            out.append(pieces)
    return out


def build_crf_nc(C=19, H=512, W=512, n_iter=5, sim_safe=False):
    assert H % P == 0 and W % P == 0
    NBH, NBW = H // P, W // P
    BW = NBH * W               # big-tile width (h-blocks side by side)
    HB = BW // 2               # half of the big tile (h-blocks 0..1)
    nc = bacc.Bacc(None, target_bir_lowering=False, debug=False)
    x0bd = nc.declare_dram_parameter("x0b", [C, H, W], BF16, isOutput=False)
    ahd = nc.declare_dram_parameter("ah", [NBH, P, H], BF16, isOutput=False)
    awd = nc.declare_dram_parameter("aw", [NBW, P, W], BF16, isOutput=False)
    idd = nc.declare_dram_parameter("ident", [P, P], BF16, isOutput=False)
    outd = nc.declare_dram_parameter("out", [C, H, W], F32, isOutput=True)

    pieces_h = _conv_pieces(NBH, H, sim_safe)
    pieces_w = _conv_pieces(NBW, W, sim_safe)

    with tile.TileContext(nc) as tc:
        with (
            tc.tile_pool(name="persist", bufs=1) as pp,
            tc.tile_pool(name="o1p", bufs=1) as o1p,
            tc.tile_pool(name="outp", bufs=3) as outp,
            tc.tile_pool(name="ps1p", bufs=2, space="PSUM") as ps1p,
            tc.tile_pool(name="ps2p", bufs=2, space="PSUM") as ps2p,
        ):
            # ---- persistent tiles ----
            ah = [pp.tile([P, H], BF16, name=f"ah{j}", tag=f"ah{j}") for j in range(NBH)]
            aw = [pp.tile([P, W], BF16, name=f"aw{j}", tag=f"aw{j}") for j in range(NBW)]
            ident = pp.tile([P, P], BF16, name="ident", tag="ident")
            nc.sync.dma_start(out=ident, in_=idd[:, :])
            for j in range(NBH):
                nc.sync.dma_start(out=ah[j], in_=ahd[j])
            for j in range(NBW):
                nc.sync.dma_start(out=aw[j], in_=awd[j])

            X0 = [pp.tile([P, BW], BF16, name=f"x0_{c}", tag=f"x0_{c}")
                  for c in range(C)]
            EM = [pp.tile([P, BW], BF16, name=f"em_{c}", tag=f"em_{c}")
                  for c in range(C)]
            pA = pp.tile([P, BW], BF16, name="pA", tag="pA")
            pB = pp.tile([P, BW], BF16, name="pB", tag="pB")
            den = pp.tile([P, BW], F32, name="den", tag="den")
            rec = pp.tile([P, BW], BF16, name="rec", tag="rec")

            O1 = {}

            def den_add(c):
                """Incremental denominator chains (pA: ch 0-9, pB: ch 10-18).

                The last channel is split in spatial halves so the softmax
                tail's half-0 chain starts as soon as exp(18, half0) lands."""
                if c == 1:
                    nc.gpsimd.tensor_add(pA, EM[0], EM[1])
                elif 2 <= c <= 9:
                    nc.gpsimd.tensor_add(pA, pA, EM[c])
                elif c == 11:
                    nc.vector.tensor_add(pB, EM[10], EM[11])
                elif 12 <= c < C - 1:
                    nc.vector.tensor_add(pB, pB, EM[c])
                elif c == C - 1:
                    for h in range(2):
                        sl = slice(h * HB, (h + 1) * HB)
                        nc.vector.tensor_add(pB[:, sl], pB[:, sl],
                                             EM[c][:, sl])

            # ---- prologue: load x0 (bf16), e_0 = exp(x0), den chains ----
            for c in range(C):
                nc.sync.dma_start(
                    out=X0[c].rearrange("p (b w) -> p b w", b=NBH),
                    in_=x0bd[c].rearrange("(b p) w -> p b w", p=P))
                nc.scalar.activation(out=EM[c], in_=X0[c], func=EXP)
                den_add(c)
                if c == 17:
                    # HAM warm-up: ~32 dummy matmuls anchored on exp(17) so
                    # the PE clock is at 2.4GHz when pass1(0) starts.
                    for k in range(2):
                        wt = ps1p.tile([P, 2 * H], F32, name="ps1", tag="ps1")
                        for i in range(16):
                            nc.tensor.matmul(
                                wt[:, i * 64:(i + 1) * 64],
                                ident, EM[17][:, 0:64],
                                start=True, stop=True)

            def mul_q(c, t=0):
                """q_c = e_c * rec in place (issued just-in-time per channel).
                In the last iteration DVE is drain-bound; late channels go to
                the otherwise-idle GPSIMD."""
                if t == n_iter - 1 and c >= 13:
                    nc.gpsimd.tensor_mul(EM[c], EM[c], rec)
                else:
                    nc.vector.tensor_mul(EM[c], EM[c], rec)

            def half_chain(h):
                """den = pA+pB; rec = 1/den (bf16) for spatial half h."""
                sl = slice(h * HB, (h + 1) * HB)
                nc.vector.tensor_add(den[:, sl], pA[:, sl], pB[:, sl])
                nc.vector.reciprocal_approx_fast(out=den[:, sl],
                                                 in_=den[:, sl])
                nc.vector.tensor_copy(out=rec[:, sl], in_=den[:, sl])

            def softmax_tail(t):
                """Iteration-boundary chain, ordered so the PE restarts fast:
                half-0 chain -> ch0 muls for the first two quarters ->
                half-1 chain -> remaining early muls. Anchor matmuls with
                staggered deps keep the PE HAM clock warm across the chain.
                Muls for c >= 3 are issued inside the channel loop."""
                wt = ps1p.tile([P, 2 * H], F32, name="ps1", tag="ps1")
                for i in range(6):
                    nc.tensor.matmul(wt[:, i * 64:(i + 1) * 64],
                                     ident, EM[C - 1][:, 0:64],
                                     start=True, stop=True)
                half_chain(0)
                for i in range(6, 12):
                    nc.tensor.matmul(wt[:, i * 64:(i + 1) * 64],
                                     ident, rec[:, 0:64],
                                     start=True, stop=True)
                for q in range(2):
                    sl = slice(q * W, (q + 1) * W)
                    nc.vector.tensor_mul(EM[0][:, sl], EM[0][:, sl],
                                         rec[:, sl])
                for i in range(12, 16):
                    nc.tensor.matmul(wt[:, i * 64:(i + 1) * 64],
                                     ident, EM[0][:, 0:64],
                                     start=True, stop=True)
                half_chain(1)
                for q in range(2, 4):
                    sl = slice(q * W, (q + 1) * W)
                    nc.vector.tensor_mul(EM[0][:, sl], EM[0][:, sl],
                                         rec[:, sl])
                mul_q(1, t)
                mul_q(2, t)

            def pass1(c, it):
                """Blur along H: o1[w,h] = sum q[h',w] Ah[h',h]; drain to SBUF."""
                o1 = o1p.tile([P, NBW * H], BF16, name="o1", tag=f"o1_{c % 2}")
                O1[c % 2] = o1
                for wcp in range(NBW // 2):
                    ps1 = ps1p.tile([P, 2 * H], F32, name="ps1", tag="ps1")
                    for wcl in range(2):
                        wc = wcp * 2 + wcl
                        prev = None
                        for j in range(NBH):
                            lhsT = EM[c][:, j * W + wc * P: j * W + wc * P + P]
                            for (lo, hi, st) in pieces_h[j]:
                                mm = nc.tensor.matmul(
                                    ps1[:, wcl * H + lo: wcl * H + hi],
                                    lhsT, ah[j][:, lo:hi],
                                    start=st,
                                    stop=(j == NBH - 1 and
                                          (lo, hi) == pieces_h[j][-1][:2]),
                                )
                                if prev is not None:
                                    add_dep_helper(mm.ins, prev.ins, sync=False,
                                                   reason="psum group order")
                                prev = mm
                    dst = o1[:, wcp * 2 * H:(wcp + 1) * 2 * H]
                    # last two channels' drains go to ACT so the DVE queue is
                    # empty when the iteration-boundary softmax chain arrives
                    if it == n_iter - 1 or c >= C - 2:
                        nc.scalar.copy(out=dst, in_=ps1)
                    elif (c + wcp) % 2 == 0:
                        nc.scalar.copy(out=dst, in_=ps1)
                    else:
                        nc.vector.tensor_copy(out=dst, in_=ps1)

            def pass2(c, it):
                """Blur along W into psum pairs; exp (or final drain + DMA)."""
                last = it == n_iter - 1
                o1 = O1[c % 2]
                for hcp in range(NBH // 2):
                    ps2 = ps2p.tile([P, 2 * W], F32, name="ps2", tag="ps2")
                    for hcl in range(2):
                        hc = hcp * 2 + hcl
                        base = hcl * W
                        prev = None
                        for j in range(NBW):
                            lhsT = o1[:, j * H + hc * P: j * H + hc * P + P]
                            for (lo, hi, st) in pieces_w[j]:
                                is_last_piece = (j == NBW - 1 and
                                                 (lo, hi) == pieces_w[j][-1][:2])
                                mm = nc.tensor.matmul(
                                    ps2[:, base + lo: base + hi],
                                    lhsT, aw[j][:, lo:hi],
                                    start=st,
                                    stop=(last and is_last_piece),
                                )
                                if prev is not None:
                                    add_dep_helper(mm.ins, prev.ins, sync=False,
                                                   reason="psum group order")
                                prev = mm
                        if not last:
                            mm = nc.tensor.matmul(
                                ps2[:, base:base + W], ident,
                                X0[c][:, hc * W:(hc + 1) * W],
                                start=False, stop=(hcl == 1))
                            add_dep_helper(mm.ins, prev.ins, sync=False,
                                           reason="psum group order")
                    if not last:
                        nc.scalar.activation(
                            out=EM[c][:, hcp * 2 * W:(hcp + 1) * 2 * W],
                            in_=ps2, func=EXP)
                    else:
                        ot = outp.tile([P, 2 * W], F32, name="ot", tag="ot")
                        nc.vector.tensor_add(
                            ot, X0[c][:, hcp * 2 * W:(hcp + 1) * 2 * W], ps2)
                        nc.sync.dma_start(
                            out=outd[c, hcp * 2 * P:(hcp + 1) * 2 * P, :]
                            .rearrange("(b p) w -> p b w", p=P),
                            in_=ot.rearrange("p (b w) -> p b w", b=2))
                if not last:
                    den_add(c)

            # ---- main loop: stagger pass2 one channel behind pass1 ----
            for t in range(n_iter):
                softmax_tail(t)
                for c in range(C):
                    if c + 3 < C:
                        mul_q(c + 3, t)
                    pass1(c, t)
                    if c > 0:
                        pass2(c - 1, t)
                pass2(C - 1, t)
    if not nc.is_finalized():
        nc.finalize()
    return nc


# ---------------- host side ----------------

def _taps(spacing, inv_theta, fs=2 * R + 1):
    d = np.float32(spacing) * np.arange(-R, R + 1, dtype=np.float32)
    k = np.exp(-np.square(d * np.float32(inv_theta)) / 2.0).astype(np.float32)
    k[R] = 0.0
    return k


def _band_matrix(k, n):
    """A[i, j] = k[i - j + R] for |i - j| <= R (out[h] = sum_h' A[h',h] q[h'])."""
    A = np.zeros((n, n), np.float32)
    for d in range(-R, R + 1):
        if k[d + R] == 0.0:
            continue
        i = np.arange(max(0, d), n + min(0, d))
        A[i, i - d] = k[d + R]
    return A


_CACHE = {}


def _get_nc():
    if "nc" not in _CACHE:
        _CACHE["nc"] = build_crf_nc()
    return _CACHE["nc"]


def make_in_maps(x, spatial_spacings, smoothness_weight, inv_smoothness_theta,
                 H=512, W=512):
    x = np.ascontiguousarray(np.asarray(x, np.float32))
    sp = np.asarray(spatial_spacings, np.float32)
    wgt = np.float32(np.asarray(smoothness_weight, np.float32))
    it = np.asarray(inv_smoothness_theta, np.float32)
    ident = np.eye(P, dtype=np.float32).astype(BF16_NP)
    in_maps = []
    for s in range(x.shape[0]):
        Ah = _band_matrix(_taps(sp[s, 0], it[0]), H)
        Aw = _band_matrix(_taps(sp[s, 1], it[1]), W) * wgt
        in_maps.append({
            "x0b": np.ascontiguousarray(x[s].astype(BF16_NP)),
            "ah": np.ascontiguousarray(Ah.reshape(H // P, P, H).astype(BF16_NP)),
            "aw": np.ascontiguousarray(Aw.reshape(W // P, P, W).astype(BF16_NP)),
            "ident": ident,
        })
    return in_maps


def kernel(x, spatial_spacings, smoothness_weight, inv_smoothness_theta):
    x = np.asarray(x, np.float32)
    assert x.shape == (8, 19, 512, 512), x.shape
    in_maps = make_in_maps(x, spatial_spacings, smoothness_weight,
                           inv_smoothness_theta)
    nc = _get_nc()
    res = run_bass_kernel_spmd(nc, in_maps, list(range(N_CORES))).results
    return np.stack([res[i]["out"] for i in range(N_CORES)]).astype(np.float32)


# revision 12
# speedup vs baseline: 1.2979x; 1.2979x over previous
"""CRF mean-field (dense_cnn) Trainium2 Bass kernel — v2 (pipelined).

Math: per iteration  x = x0 + w*separable_blur(softmax(x, axis=C))
(the -I compatibility einsum is a sign flip). Core i handles sample i.

Per-core scheme (SBUF-resident, bf16 compute):
  X0[c]  [128, 2048] bf16   x0, h-blocks side by side (host-cast)
  EM[c]  [128, 2048] bf16   e = exp(x0+m); softmax turns it into q in place
  Blur on PE with banded bf16 matrices Ah/Aw (weight folded into Aw):
    pass1: o1[w,h] = sum_h' q[h',w] Ah[h',h]   (lhsT=q block, rhs=Ah)
    pass2: m[h,w]  = sum_w' o1[w',h] Aw[w',w]  (lhsT=o1 block, rhs=Aw)
  x0 rides the PE via identity matmuls into the pass-2 PSUM (iters 0..3),
  so ACT's exp(psum) directly yields e for the next iteration.

v2 scheduling (vs v1):
  - Channel software pipelining: pass1(c+1) is issued between pass1(c)'s
    copies and pass2(c), so the PE never stalls on the PSUM->SBUF drains.
  - o1 drains alternate ACT/DVE; exp done as 2x[128,1024] (psum pair).
  - den accumulated in two chains (pA ch0-9, pB ch10-18) interleaved with
    the channel loop; softmax tail (merge/recip/cast) split in spatial
    halves and first-channel muls in quarters so the PE restart after an
    iteration boundary stays under the HAM re-throttle window.
  - Last iteration: no identity matmuls; drain is DVE add of bf16 x0 with
    the psum (no f32 x0 reload), output DMA per [128,1024] half.
  - Dummy warm-up matmuls (anchored on exp(17) of the prologue) bring the
    PE HAM clock to 2.4GHz before the first real pass1.
"""

import sys

for _p in ("/opt/trn_rl_repo",):
    if _p not in sys.path:
        sys.path.insert(0, _p)

import numpy as np
import ml_dtypes

import concourse.bass as bass
from concourse import bacc
import concourse.mybir as mybir
import concourse.tile as tile
from concourse.bass_utils import run_bass_kernel_spmd
from concourse.tile_rust import add_dep_helper

F32 = mybir.dt.float32
BF16 = mybir.dt.bfloat16
P = 128
R = 5          # filter radius (FS=11)
N_CORES = 8

BF16_NP = ml_dtypes.bfloat16

EXP = mybir.ActivationFunctionType.Exp


def _conv_pieces(nb, n, sim_safe):
    """Per row-block j, list of (lo, hi, is_start) output-column windows."""
    out = []
    for j in range(nb):
        base = j * P
        hi = min(n, base + P + R)
        if j == 0:
            out.append([(0, hi, True)])
        elif not sim_safe:
            out.append([(base - R, hi, False)])
        else:
            pieces = [(base - R, base + R, False)]       # overlap: accumulate
            if base + R < hi:
                pieces.append((base + R, hi, False))     # fresh: overwrite
            out.append(pieces)
    return out


def build_crf_nc(C=19, H=512, W=512, n_iter=5, sim_safe=False):
    assert H % P == 0 and W % P == 0
    NBH, NBW = H // P, W // P
    BW = NBH * W               # big-tile width (h-blocks side by side)
    HB = BW // 2               # half of the big tile (h-blocks 0..1)
    nc = bacc.Bacc(None, target_bir_lowering=False, debug=False)
    x0bd = nc.declare_dram_parameter("x0b", [C, H, W], BF16, isOutput=False)
    ahd = nc.declare_dram_parameter("ah", [NBH, P, H], BF16, isOutput=False)
    awd = nc.declare_dram_parameter("aw", [NBW, P, W], BF16, isOutput=False)
    idd = nc.declare_dram_parameter("ident", [P, P], BF16, isOutput=False)
    outd = nc.declare_dram_parameter("out", [C, H, W], F32, isOutput=True)

    pieces_h = _conv_pieces(NBH, H, sim_safe)
    pieces_w = _conv_pieces(NBW, W, sim_safe)

    with tile.TileContext(nc) as tc:
        with (
            tc.tile_pool(name="persist", bufs=1) as pp,
            tc.tile_pool(name="o1p", bufs=1) as o1p,
            tc.tile_pool(name="outp", bufs=3) as outp,
            tc.tile_pool(name="ps1p", bufs=2, space="PSUM") as ps1p,
            tc.tile_pool(name="ps2p", bufs=2, space="PSUM") as ps2p,
        ):
            # ---- persistent tiles ----
            ah = [pp.tile([P, H], BF16, name=f"ah{j}", tag=f"ah{j}") for j in range(NBH)]
            aw = [pp.tile([P, W], BF16, name=f"aw{j}", tag=f"aw{j}") for j in range(NBW)]
            ident = pp.tile([P, P], BF16, name="ident", tag="ident")
            nc.sync.dma_start(out=ident, in_=idd[:, :])
            for j in range(NBH):
                nc.sync.dma_start(out=ah[j], in_=ahd[j])
            for j in range(NBW):
                nc.sync.dma_start(out=aw[j], in_=awd[j])

            X0 = [pp.tile([P, BW], BF16, name=f"x0_{c}", tag=f"x0_{c}")
                  for c in range(C)]
            EM = [pp.tile([P, BW], BF16, name=f"em_{c}", tag=f"em_{c}")
                  for c in range(C)]
            pA = pp.tile([P, BW], BF16, name="pA", tag="pA")
            pB = pp.tile([P, BW], BF16, name="pB", tag="pB")
            den = pp.tile([P, BW], F32, name="den", tag="den")
            rec = pp.tile([P, BW], BF16, name="rec", tag="rec")

            O1 = {}

            def den_add(c):
                """Incremental denominator chains (pA: ch 0-9, pB: ch 10-18).

                The last channel is split in spatial halves so the softmax
                tail's half-0 chain starts as soon as exp(18, half0) lands."""
                if c == 1:
                    nc.vector.tensor_add(pA, EM[0], EM[1])
                elif 2 <= c <= 9:
                    nc.vector.tensor_add(pA, pA, EM[c])
                elif c == 11:
                    nc.vector.tensor_add(pB, EM[10], EM[11])
                elif 12 <= c < C - 1:
                    nc.vector.tensor_add(pB, pB, EM[c])
                elif c == C - 1:
                    for h in range(2):
                        sl = slice(h * HB, (h + 1) * HB)
                        nc.vector.tensor_add(pB[:, sl], pB[:, sl],
                                             EM[c][:, sl])

            # ---- prologue: load x0 (bf16), e_0 = exp(x0), den chains ----
            for c in range(C):
                nc.sync.dma_start(
                    out=X0[c].rearrange("p (b w) -> p b w", b=NBH),
                    in_=x0bd[c].rearrange("(b p) w -> p b w", p=P))
                nc.scalar.activation(out=EM[c], in_=X0[c], func=EXP)
                den_add(c)
                if c == 17:
                    # HAM warm-up: ~32 dummy matmuls anchored on exp(17) so
                    # the PE clock is at 2.4GHz when pass1(0) starts.
                    for k in range(2):
                        wt = ps1p.tile([P, 2 * H], F32, name="ps1", tag="ps1")
                        for i in range(16):
                            nc.tensor.matmul(
                                wt[:, i * 64:(i + 1) * 64],
                                ident, EM[17][:, 0:64],
                                start=True, stop=True)

            def mul_q(c, t=0):
                """q_c = e_c * rec in place (issued just-in-time per channel)."""
                nc.vector.tensor_mul(EM[c], EM[c], rec)

            def half_chain(h):
                """den = pA+pB; rec = 1/den (bf16) for spatial half h."""
                sl = slice(h * HB, (h + 1) * HB)
                nc.vector.tensor_add(den[:, sl], pA[:, sl], pB[:, sl])
                nc.vector.reciprocal_approx_fast(out=den[:, sl],
                                                 in_=den[:, sl])
                nc.vector.tensor_copy(out=rec[:, sl], in_=den[:, sl])

            def softmax_tail(t):
                """Iteration-boundary chain, ordered so the PE restarts fast:
                half-0 chain -> ch0 muls for the first two quarters ->
                half-1 chain -> remaining early muls. Anchor matmuls with
                staggered deps keep the PE HAM clock warm across the chain.
                Muls for c >= 3 are issued inside the channel loop."""
                wt = ps1p.tile([P, 2 * H], F32, name="ps1", tag="ps1")
                for i in range(6):
                    nc.tensor.matmul(wt[:, i * 64:(i + 1) * 64],
                                     ident, EM[C - 1][:, 0:64],
                                     start=True, stop=True)
                half_chain(0)
                for i in range(6, 12):
                    nc.tensor.matmul(wt[:, i * 64:(i + 1) * 64],
                                     ident, rec[:, 0:64],
                                     start=True, stop=True)
                for q in range(2):
                    sl = slice(q * W, (q + 1) * W)
                    nc.vector.tensor_mul(EM[0][:, sl], EM[0][:, sl],
                                         rec[:, sl])
                for i in range(12, 16):
                    nc.tensor.matmul(wt[:, i * 64:(i + 1) * 64],
                                     ident, EM[0][:, 0:64],
                                     start=True, stop=True)
                half_chain(1)
                for q in range(2, 4):
                    sl = slice(q * W, (q + 1) * W)
                    nc.vector.tensor_mul(EM[0][:, sl], EM[0][:, sl],
                                         rec[:, sl])
                mul_q(1, t)
                mul_q(2, t)

            def pass1(c, it):
                """Blur along H: o1[w,h] = sum q[h',w] Ah[h',h]; drain to SBUF."""
                o1 = o1p.tile([P, NBW * H], BF16, name="o1", tag=f"o1_{c % 2}")
                O1[c % 2] = o1
                for wcp in range(NBW // 2):
                    ps1 = ps1p.tile([P, 2 * H], F32, name="ps1", tag="ps1")
                    for wcl in range(2):
                        wc = wcp * 2 + wcl
                        prev = None
                        for j in range(NBH):
                            lhsT = EM[c][:, j * W + wc * P: j * W + wc * P + P]
                            for (lo, hi, st) in pieces_h[j]:
                                mm = nc.tensor.matmul(
                                    ps1[:, wcl * H + lo: wcl * H + hi],
                                    lhsT, ah[j][:, lo:hi],
                                    start=st,
                                    stop=(j == NBH - 1 and
                                          (lo, hi) == pieces_h[j][-1][:2]),
                                )
                                if prev is not None:
                                    add_dep_helper(mm.ins, prev.ins, sync=False,
                                                   reason="psum group order")
                                prev = mm
                    dst = o1[:, wcp * 2 * H:(wcp + 1) * 2 * H]
                    # last two channels' drains go to ACT so the DVE queue is
                    # empty when the iteration-boundary softmax chain arrives
                    if it == n_iter - 1 or c >= C - 2:
                        nc.scalar.copy(out=dst, in_=ps1)
                    elif (c + wcp) % 2 == 0:
                        nc.scalar.copy(out=dst, in_=ps1)
                    else:
                        nc.vector.tensor_copy(out=dst, in_=ps1)

            def pass2(c, it):
                """Blur along W into psum pairs; exp (or final drain + DMA)."""
                last = it == n_iter - 1
                o1 = O1[c % 2]
                for hcp in range(NBH // 2):
                    ps2 = ps2p.tile([P, 2 * W], F32, name="ps2", tag="ps2")
                    for hcl in range(2):
                        hc = hcp * 2 + hcl
                        base = hcl * W
                        prev = None
                        for j in range(NBW):
                            lhsT = o1[:, j * H + hc * P: j * H + hc * P + P]
                            for (lo, hi, st) in pieces_w[j]:
                                is_last_piece = (j == NBW - 1 and
                                                 (lo, hi) == pieces_w[j][-1][:2])
                                mm = nc.tensor.matmul(
                                    ps2[:, base + lo: base + hi],
                                    lhsT, aw[j][:, lo:hi],
                                    start=st,
                                    stop=(last and is_last_piece),
                                )
                                if prev is not None:
                                    add_dep_helper(mm.ins, prev.ins, sync=False,
                                                   reason="psum group order")
                                prev = mm
                        if not last:
                            mm = nc.tensor.matmul(
                                ps2[:, base:base + W], ident,
                                X0[c][:, hc * W:(hc + 1) * W],
                                start=False, stop=(hcl == 1))
                            add_dep_helper(mm.ins, prev.ins, sync=False,
                                           reason="psum group order")
                    if not last:
                        nc.scalar.activation(
                            out=EM[c][:, hcp * 2 * W:(hcp + 1) * 2 * W],
                            in_=ps2, func=EXP)
                    else:
                        ot = outp.tile([P, 2 * W], F32, name="ot", tag="ot")
                        nc.vector.tensor_add(
                            ot, X0[c][:, hcp * 2 * W:(hcp + 1) * 2 * W], ps2)
                        nc.sync.dma_start(
                            out=outd[c, hcp * 2 * P:(hcp + 1) * 2 * P, :]
                            .rearrange("(b p) w -> p b w", p=P),
                            in_=ot.rearrange("p (b w) -> p b w", b=2))
                if not last:
                    den_add(c)

            # ---- main loop: stagger pass2 one channel behind pass1 ----
            for t in range(n_iter):
                softmax_tail(t)
                for c in range(C):
                    if c + 3 < C:
                        mul_q(c + 3, t)
                    pass1(c, t)
                    if c > 0:
                        pass2(c - 1, t)
                pass2(C - 1, t)
    if not nc.is_finalized():
        nc.finalize()
    return nc


# ---------------- host side ----------------

def _taps(spacing, inv_theta, fs=2 * R + 1):
    d = np.float32(spacing) * np.arange(-R, R + 1, dtype=np.float32)
    k = np.exp(-np.square(d * np.float32(inv_theta)) / 2.0).astype(np.float32)
    k[R] = 0.0
    return k


def _band_matrix(k, n):
    """A[i, j] = k[i - j + R] for |i - j| <= R (out[h] = sum_h' A[h',h] q[h'])."""
    A = np.zeros((n, n), np.float32)
    for d in range(-R, R + 1):
        if k[d + R] == 0.0:
            continue
        i = np.arange(max(0, d), n + min(0, d))
        A[i, i - d] = k[d + R]
    return A


_CACHE = {}


def _get_nc():
    if "nc" not in _CACHE:
        _CACHE["nc"] = build_crf_nc()
    return _CACHE["nc"]


def make_in_maps(x, spatial_spacings, smoothness_weight, inv_smoothness_theta,
                 H=512, W=512):
    x = np.ascontiguousarray(np.asarray(x, np.float32))
    sp = np.asarray(spatial_spacings, np.float32)
    wgt = np.float32(np.asarray(smoothness_weight, np.float32))
    it = np.asarray(inv_smoothness_theta, np.float32)
    ident = np.eye(P, dtype=np.float32).astype(BF16_NP)
    in_maps = []
    for s in range(x.shape[0]):
        Ah = _band_matrix(_taps(sp[s, 0], it[0]), H)
        Aw = _band_matrix(_taps(sp[s, 1], it[1]), W) * wgt
        in_maps.append({
            "x0b": np.ascontiguousarray(x[s].astype(BF16_NP)),
            "ah": np.ascontiguousarray(Ah.reshape(H // P, P, H).astype(BF16_NP)),
            "aw": np.ascontiguousarray(Aw.reshape(W // P, P, W).astype(BF16_NP)),
            "ident": ident,
        })
    return in_maps


def kernel(x, spatial_spacings, smoothness_weight, inv_smoothness_theta):
    x = np.asarray(x, np.float32)
    assert x.shape == (8, 19, 512, 512), x.shape
    in_maps = make_in_maps(x, spatial_spacings, smoothness_weight,
                           inv_smoothness_theta)
    nc = _get_nc()
    res = run_bass_kernel_spmd(nc, in_maps, list(range(N_CORES))).results
    return np.stack([res[i]["out"] for i in range(N_CORES)]).astype(np.float32)


# revision 15
# speedup vs baseline: 1.2999x; 1.0016x over previous
"""CRF mean-field (dense_cnn) Trainium2 Bass kernel — v2 (pipelined).

Math: per iteration  x = x0 + w*separable_blur(softmax(x, axis=C))
(the -I compatibility einsum is a sign flip). Core i handles sample i.

Per-core scheme (SBUF-resident, bf16 compute):
  X0[c]  [128, 2048] bf16   x0, h-blocks side by side (host-cast)
  EM[c]  [128, 2048] bf16   e = exp(x0+m); softmax turns it into q in place
  Blur on PE with banded bf16 matrices Ah/Aw (weight folded into Aw):
    pass1: o1[w,h] = sum_h' q[h',w] Ah[h',h]   (lhsT=q block, rhs=Ah)
    pass2: m[h,w]  = sum_w' o1[w',h] Aw[w',w]  (lhsT=o1 block, rhs=Aw)
  x0 rides the PE via identity matmuls into the pass-2 PSUM (iters 0..3),
  so ACT's exp(psum) directly yields e for the next iteration.

v2 scheduling (vs v1):
  - Channel software pipelining: pass1(c+1) is issued between pass1(c)'s
    copies and pass2(c), so the PE never stalls on the PSUM->SBUF drains.
  - o1 drains alternate ACT/DVE; exp done as 2x[128,1024] (psum pair).
  - den accumulated in two chains (pA ch0-9, pB ch10-18) interleaved with
    the channel loop; softmax tail (merge/recip/cast) split in spatial
    halves and first-channel muls in quarters so the PE restart after an
    iteration boundary stays under the HAM re-throttle window.
  - Last iteration: no identity matmuls; drain is DVE add of bf16 x0 with
    the psum (no f32 x0 reload), output DMA per [128,1024] half.
  - Dummy warm-up matmuls (anchored on exp(17) of the prologue) bring the
    PE HAM clock to 2.4GHz before the first real pass1.
"""

import sys

for _p in ("/opt/trn_rl_repo",):
    if _p not in sys.path:
        sys.path.insert(0, _p)

import numpy as np
import ml_dtypes

import concourse.bass as bass
from concourse import bacc
import concourse.mybir as mybir
import concourse.tile as tile
from concourse.bass_utils import run_bass_kernel_spmd
from concourse.tile_rust import add_dep_helper

F32 = mybir.dt.float32
BF16 = mybir.dt.bfloat16
P = 128
R = 5          # filter radius (FS=11)
N_CORES = 8

BF16_NP = ml_dtypes.bfloat16

EXP = mybir.ActivationFunctionType.Exp


def _conv_pieces(nb, n, sim_safe):
    """Per row-block j, list of (lo, hi, is_start) output-column windows."""
    out = []
    for j in range(nb):
        base = j * P
        hi = min(n, base + P + R)
        if j == 0:
            out.append([(0, hi, True)])
        elif not sim_safe:
            out.append([(base - R, hi, False)])
        else:
            pieces = [(base - R, base + R, False)]       # overlap: accumulate
            if base + R < hi:
                pieces.append((base + R, hi, False))     # fresh: overwrite
            out.append(pieces)
    return out


def build_crf_nc(C=19, H=512, W=512, n_iter=5, sim_safe=False):
    assert H % P == 0 and W % P == 0
    NBH, NBW = H // P, W // P
    BW = NBH * W               # big-tile width (h-blocks side by side)
    HB = BW // 2               # half of the big tile (h-blocks 0..1)
    nc = bacc.Bacc(None, target_bir_lowering=False, debug=False)
    x0bd = nc.declare_dram_parameter("x0b", [C, H, W], BF16, isOutput=False)
    ahd = nc.declare_dram_parameter("ah", [NBH, P, H], BF16, isOutput=False)
    awd = nc.declare_dram_parameter("aw", [NBW, P, W], BF16, isOutput=False)
    idd = nc.declare_dram_parameter("ident", [P, P], BF16, isOutput=False)
    outd = nc.declare_dram_parameter("out", [C, H, W], F32, isOutput=True)

    pieces_h = _conv_pieces(NBH, H, sim_safe)
    pieces_w = _conv_pieces(NBW, W, sim_safe)

    with tile.TileContext(nc) as tc:
        with (
            tc.tile_pool(name="persist", bufs=1) as pp,
            tc.tile_pool(name="o1p", bufs=1) as o1p,
            tc.tile_pool(name="outp", bufs=3) as outp,
            tc.tile_pool(name="ps1p", bufs=2, space="PSUM") as ps1p,
            tc.tile_pool(name="ps2p", bufs=2, space="PSUM") as ps2p,
        ):
            # ---- persistent tiles ----
            ah = [pp.tile([P, H], BF16, name=f"ah{j}", tag=f"ah{j}") for j in range(NBH)]
            aw = [pp.tile([P, W], BF16, name=f"aw{j}", tag=f"aw{j}") for j in range(NBW)]
            ident = pp.tile([P, P], BF16, name="ident", tag="ident")
            nc.sync.dma_start(out=ident, in_=idd[:, :])
            for j in range(NBH):
                nc.sync.dma_start(out=ah[j], in_=ahd[j])
            for j in range(NBW):
                nc.sync.dma_start(out=aw[j], in_=awd[j])

            X0 = [pp.tile([P, BW], BF16, name=f"x0_{c}", tag=f"x0_{c}")
                  for c in range(C)]
            EM = [pp.tile([P, BW], BF16, name=f"em_{c}", tag=f"em_{c}")
                  for c in range(C)]
            pA = pp.tile([P, BW], BF16, name="pA", tag="pA")
            pB = pp.tile([P, BW], BF16, name="pB", tag="pB")
            den = pp.tile([P, BW], F32, name="den", tag="den")
            rec = pp.tile([P, BW], BF16, name="rec", tag="rec")

            O1 = {}

            def den_add(c):
                """Incremental denominator chains (pA: ch 0-9, pB: ch 10-18).

                The last channel is split in spatial halves so the softmax
                tail's half-0 chain starts as soon as exp(18, half0) lands."""
                if c == 1:
                    nc.vector.tensor_add(pA, EM[0], EM[1])
                elif 2 <= c <= 9:
                    nc.vector.tensor_add(pA, pA, EM[c])
                elif c == 11:
                    nc.vector.tensor_add(pB, EM[10], EM[11])
                elif 12 <= c < C - 1:
                    nc.vector.tensor_add(pB, pB, EM[c])
                elif c == C - 1:
                    for q in range(4):
                        sl = slice(q * W, (q + 1) * W)
                        nc.vector.tensor_add(pB[:, sl], pB[:, sl],
                                             EM[c][:, sl])

            # ---- prologue: load x0 (bf16), e_0 = exp(x0), den chains ----
            for c in range(C):
                nc.sync.dma_start(
                    out=X0[c].rearrange("p (b w) -> p b w", b=NBH),
                    in_=x0bd[c].rearrange("(b p) w -> p b w", p=P))
                nc.scalar.activation(out=EM[c], in_=X0[c], func=EXP)
                den_add(c)
                if c == 17:
                    # HAM warm-up: ~32 dummy matmuls anchored on exp(17) so
                    # the PE clock is at 2.4GHz when pass1(0) starts.
                    for k in range(2):
                        wt = ps1p.tile([P, 2 * H], F32, name="ps1", tag="ps1")
                        for i in range(16):
                            nc.tensor.matmul(
                                wt[:, i * 64:(i + 1) * 64],
                                ident, EM[17][:, 0:64],
                                start=True, stop=True)

            def mul_q(c, t=0):
                """q_c = e_c * rec in place (issued just-in-time per channel)."""
                nc.vector.tensor_mul(EM[c], EM[c], rec)

            def quarter_chain(q):
                """den = pA+pB; rec = 1/den (bf16); q0 = e0*rec, spatial
                quarter q — the minimal chain that unblocks pass1(0) MMs."""
                sl = slice(q * W, (q + 1) * W)
                nc.vector.tensor_add(den[:, sl], pA[:, sl], pB[:, sl])
                nc.vector.reciprocal_approx_fast(out=den[:, sl],
                                                 in_=den[:, sl])
                nc.vector.tensor_copy(out=rec[:, sl], in_=den[:, sl])
                nc.vector.tensor_mul(EM[0][:, sl], EM[0][:, sl], rec[:, sl])

            def softmax_tail(t):
                """Iteration-boundary chain in spatial quarters so the PE
                restarts quickly; anchor matmuls with staggered deps keep
                the PE HAM clock warm across the chain. Muls for c >= 3 are
                issued inside the channel loop."""
                wt = ps1p.tile([P, 2 * H], F32, name="ps1", tag="ps1")
                for i in range(5):
                    nc.tensor.matmul(wt[:, i * 64:(i + 1) * 64],
                                     ident, EM[C - 1][:, 0:64],
                                     start=True, stop=True)
                quarter_chain(0)
                for i in range(5, 10):
                    nc.tensor.matmul(wt[:, i * 64:(i + 1) * 64],
                                     ident, rec[:, 0:64],
                                     start=True, stop=True)
                quarter_chain(1)
                for i in range(10, 14):
                    nc.tensor.matmul(wt[:, i * 64:(i + 1) * 64],
                                     ident, EM[0][:, 0:64],
                                     start=True, stop=True)
                quarter_chain(2)
                quarter_chain(3)
                for q in range(4):
                    sl = slice(q * W, (q + 1) * W)
                    nc.vector.tensor_mul(EM[1][:, sl], EM[1][:, sl],
                                         rec[:, sl])
                mul_q(2, t)

            def pass1(c, it):
                """Blur along H: o1[w,h] = sum q[h',w] Ah[h',h]; drain to SBUF."""
                o1 = o1p.tile([P, NBW * H], BF16, name="o1", tag=f"o1_{c % 2}")
                O1[c % 2] = o1
                for wcp in range(NBW // 2):
                    ps1 = ps1p.tile([P, 2 * H], F32, name="ps1", tag="ps1")
                    for wcl in range(2):
                        wc = wcp * 2 + wcl
                        prev = None
                        for j in range(NBH):
                            lhsT = EM[c][:, j * W + wc * P: j * W + wc * P + P]
                            for (lo, hi, st) in pieces_h[j]:
                                mm = nc.tensor.matmul(
                                    ps1[:, wcl * H + lo: wcl * H + hi],
                                    lhsT, ah[j][:, lo:hi],
                                    start=st,
                                    stop=(j == NBH - 1 and
                                          (lo, hi) == pieces_h[j][-1][:2]),
                                )
                                if prev is not None:
                                    add_dep_helper(mm.ins, prev.ins, sync=False,
                                                   reason="psum group order")
                                prev = mm
                    dst = o1[:, wcp * 2 * H:(wcp + 1) * 2 * H]
                    # last two channels' drains go to ACT so the DVE queue is
                    # empty when the iteration-boundary softmax chain arrives
                    if it == n_iter - 1 or c >= C - 2:
                        nc.scalar.copy(out=dst, in_=ps1)
                    elif (c + wcp) % 2 == 0:
                        nc.scalar.copy(out=dst, in_=ps1)
                    else:
                        nc.vector.tensor_copy(out=dst, in_=ps1)

            def pass2(c, it):
                """Blur along W into psum pairs; exp (or final drain + DMA).

                In the last iteration the x0 add rides the PE (identity
                matmuls) only for the first few channels, whose drains are
                then plain ACT copies; the rest add x0 on DVE — this
                balances ACT/DVE/PE in the exp-free final iteration."""
                last = it == n_iter - 1
                use_ident = (not last) or c < 5
                o1 = O1[c % 2]
                for hcp in range(NBH // 2):
                    ps2 = ps2p.tile([P, 2 * W], F32, name="ps2", tag="ps2")
                    for hcl in range(2):
                        hc = hcp * 2 + hcl
                        base = hcl * W
                        prev = None
                        for j in range(NBW):
                            lhsT = o1[:, j * H + hc * P: j * H + hc * P + P]
                            for (lo, hi, st) in pieces_w[j]:
                                is_last_piece = (j == NBW - 1 and
                                                 (lo, hi) == pieces_w[j][-1][:2])
                                mm = nc.tensor.matmul(
                                    ps2[:, base + lo: base + hi],
                                    lhsT, aw[j][:, lo:hi],
                                    start=st,
                                    stop=(not use_ident and is_last_piece),
                                )
                                if prev is not None:
                                    add_dep_helper(mm.ins, prev.ins, sync=False,
                                                   reason="psum group order")
                                prev = mm
                        if use_ident:
                            mm = nc.tensor.matmul(
                                ps2[:, base:base + W], ident,
                                X0[c][:, hc * W:(hc + 1) * W],
                                start=False, stop=(hcl == 1))
                            add_dep_helper(mm.ins, prev.ins, sync=False,
                                           reason="psum group order")
                    if not last:
                        nc.scalar.activation(
                            out=EM[c][:, hcp * 2 * W:(hcp + 1) * 2 * W],
                            in_=ps2, func=EXP)
                    else:
                        ot = outp.tile([P, 2 * W], F32, name="ot", tag="ot")
                        if use_ident:
                            nc.scalar.copy(out=ot, in_=ps2)
                        else:
                            nc.vector.tensor_add(
                                ot, X0[c][:, hcp * 2 * W:(hcp + 1) * 2 * W],
                                ps2)
                        nc.sync.dma_start(
                            out=outd[c, hcp * 2 * P:(hcp + 1) * 2 * P, :]
                            .rearrange("(b p) w -> p b w", p=P),
                            in_=ot.rearrange("p (b w) -> p b w", b=2))
                if not last:
                    den_add(c)

            # ---- main loop: stagger pass2 one channel behind pass1 ----
            for t in range(n_iter):
                softmax_tail(t)
                for c in range(C):
                    if c + 3 < C:
                        mul_q(c + 3, t)
                    pass1(c, t)
                    if c > 0:
                        pass2(c - 1, t)
                pass2(C - 1, t)
    if not nc.is_finalized():
        nc.finalize()
    return nc


# ---------------- host side ----------------

def _taps(spacing, inv_theta, fs=2 * R + 1):
    d = np.float32(spacing) * np.arange(-R, R + 1, dtype=np.float32)
    k = np.exp(-np.square(d * np.float32(inv_theta)) / 2.0).astype(np.float32)
    k[R] = 0.0
    return k


def _band_matrix(k, n):
    """A[i, j] = k[i - j + R] for |i - j| <= R (out[h] = sum_h' A[h',h] q[h'])."""
    A = np.zeros((n, n), np.float32)
    for d in range(-R, R + 1):
        if k[d + R] == 0.0:
            continue
        i = np.arange(max(0, d), n + min(0, d))
        A[i, i - d] = k[d + R]
    return A


_CACHE = {}


def _get_nc():
    if "nc" not in _CACHE:
        _CACHE["nc"] = build_crf_nc()
    return _CACHE["nc"]


def make_in_maps(x, spatial_spacings, smoothness_weight, inv_smoothness_theta,
                 H=512, W=512):
    x = np.ascontiguousarray(np.asarray(x, np.float32))
    sp = np.asarray(spatial_spacings, np.float32)
    wgt = np.float32(np.asarray(smoothness_weight, np.float32))
    it = np.asarray(inv_smoothness_theta, np.float32)
    ident = np.eye(P, dtype=np.float32).astype(BF16_NP)
    in_maps = []
    for s in range(x.shape[0]):
        Ah = _band_matrix(_taps(sp[s, 0], it[0]), H)
        Aw = _band_matrix(_taps(sp[s, 1], it[1]), W) * wgt
        in_maps.append({
            "x0b": np.ascontiguousarray(x[s].astype(BF16_NP)),
            "ah": np.ascontiguousarray(Ah.reshape(H // P, P, H).astype(BF16_NP)),
            "aw": np.ascontiguousarray(Aw.reshape(W // P, P, W).astype(BF16_NP)),
            "ident": ident,
        })
    return in_maps


def kernel(x, spatial_spacings, smoothness_weight, inv_smoothness_theta):
    x = np.asarray(x, np.float32)
    assert x.shape == (8, 19, 512, 512), x.shape
    in_maps = make_in_maps(x, spatial_spacings, smoothness_weight,
                           inv_smoothness_theta)
    nc = _get_nc()
    res = run_bass_kernel_spmd(nc, in_maps, list(range(N_CORES))).results
    return np.stack([res[i]["out"] for i in range(N_CORES)]).astype(np.float32)
